# revision 12
# baseline (speedup 1.0000x reference)
"""Trainium2 Bass kernel for nn_EnhancedFinancialGAT.

Mathematical collapse: the reference broadcasts each batch item's feature
vector to all N=2000 graph nodes, so every node starts identical. A GAT
layer on identical node features returns, for every node, the attention-
weighted average of identical projected vectors -- and per-dst softmax
weights sum to exactly 1 in f32 (denom + 1e-16 == denom), so each layer
reduces to relu(h @ W.T + b). Every node stays identical through all 3
layers, and the company-node gather picks that shared vector. The whole
model is therefore an MLP:

  h = relu(x @ W_in.T + b_in)
  h = relu(h @ gat_W[l].T + gat_b[l])   for l in 0..2
  fused = relu(concat([h, emb[company_indices]]) @ W_fuse.T + b_fuse)
  price = W_p3 @ relu(W_p2 @ relu(W_p1 @ fused + b_p1) + b_p2) + b_p3
  direction = sigmoid(same with d-weights)

Verified numerically: collapsed-vs-full relative error ~2e-7 (pure f32
rounding noise of the softmax-weighted sums).

Sharding: data-parallel over batch (64 rows -> 8 rows/core). Weights are
replicated, pre-transposed on host into one packed fp16 [128, COLS] DRAM
tensor per core (activations-transposed layout [feature, batch], so no
on-device transposes are needed). fp16 matmul inputs with fp32 PSUM
accumulation and fp32 biases keep the end-to-end error ~1e-4.
"""

import numpy as np

USE_F16 = False  # fp32 is exact (6e-7); fp16 lands ~1e-3 (PE computes 16-bit at bf16 precision)

B = 64
N_CORES = 8
BPC = B // N_CORES  # batch rows per core

# -------- packed fp16 [128, COLS] layout (column offsets) --------
OFF_ACTS = 0                       # 16 cols: xT 0:8, embT 8:16 (rows 0:64, dup to 64:128)
OFF_WIN = 16                       # 128 cols: rows 0:64 -> M[0:128], rows 64:128 -> M[128:256]
OFF_GAT = 144                      # 6 blocks of 256 cols: block (l,k) at (2l+k)*256
OFF_FUSE = OFF_GAT + 6 * 256       # k0 [128,256], k1 [128,256]
OFF_FUSE2 = OFF_FUSE + 512         # k2 row-split, 128 cols
OFF_P1 = OFF_FUSE2 + 128           # k0 cols 0:128, k1 cols 128:256
OFF_D1 = OFF_P1 + 256
OFF_P2 = OFF_D1 + 256              # [128, 64]
OFF_D2 = OFF_P2 + 64
OFF_P3 = OFF_D2 + 64               # rows 0:64, 1 col
OFF_D3 = OFF_P3 + 1
COLS = OFF_D3 + 1                  # 2962

# fp32 bias tensor [128, 16] column map:
#  0,1: b_in | 2..7: gat_b (l,m) | 8,9: b_fuse | 10: b_p1 | 11: b_d1
#  12: b_p2 (rows 0:64) | 13: b_d2 | 14 row0: b_p3 | 15 row0: b_d3

_CACHE = {}

RAW = True  # raw-Bass build (manual semaphores, no Tile preamble/tail barriers)


def _build_nc():
    return _build_nc_raw() if RAW else _build_nc_tile()


def _build_nc_raw():
    """Raw Bass: explicit per-engine programs + semaphores.

    Avoids TileContext's kernel-tail drain + double all-engine barrier and
    its per-instruction attached waits. PSUM: 4 rotating [128,8] banks for
    the wide groups (WAR covered by the RAW waits, since DVE is in-order)
    + 4 dedicated banks for the head tails = exactly 8.
    """
    from contextlib import ExitStack

    import concourse.bass as bass
    import concourse.mybir as mybir

    f32 = mybir.dt.float32
    f16 = mybir.dt.float16 if USE_F16 else mybir.dt.float32
    ADD = mybir.AluOpType.add
    MAX = mybir.AluOpType.max

    nc = bass.Bass("TRN2", debug=False, num_devices=N_CORES)
    pack = nc.declare_dram_parameter("pack", [128, COLS], f16, isOutput=False)
    biasd = nc.declare_dram_parameter("biasp", [128, 16], f32, isOutput=False)
    out_d = nc.declare_dram_parameter("out", [2, BPC], f32, isOutput=True)

    chunks = [0, OFF_GAT, OFF_GAT + 512, OFF_GAT + 1024, OFF_FUSE, OFF_P1, COLS]

    ctx = ExitStack()
    with ctx:
        sb = lambda nm, shape, dt: ctx.enter_context(nc.sbuf_tensor(nm, shape, dt))
        psb = lambda nm, shape: ctx.enter_context(nc.psum_tensor(nm, shape, f32))
        W = sb("W", [128, COLS], f16)
        Bt = sb("Bt", [128, 16], f32)
        # activation tiles (persistent, SBUF is plentiful)
        h0 = [sb(f"h0_{i}", [128, BPC], f16) for i in range(2)]
        g1 = [sb(f"g1_{i}", [128, BPC], f16) for i in range(2)]
        g2 = [sb(f"g2_{i}", [128, BPC], f16) for i in range(2)]
        g3 = [sb(f"g3_{i}", [128, BPC], f16) for i in range(2)]
        fu = [sb(f"fu_{i}", [128, BPC], f16) for i in range(2)]
        a1p = sb("a1p", [128, BPC], f16)
        a1d = sb("a1d", [128, BPC], f16)
        a2p = sb("a2p", [64, BPC], f16)
        a2d = sb("a2d", [64, BPC], f16)
        price = sb("price", [1, BPC], f32)
        dirn = sb("dirn", [1, BPC], f32)
        # PSUM: exactly 8 banks
        A = [psb(f"A{i}", [128, BPC]) for i in range(4)]
        Pp2 = psb("Pp2", [64, BPC])
        Pd2 = psb("Pd2", [64, BPC])
        Pp3 = psb("Pp3", [1, BPC])
        Pd3 = psb("Pd3", [1, BPC])

        csem = [ctx.enter_context(nc.semaphore(f"c{i}")) for i in range(6)]
        bias_sem = ctx.enter_context(nc.semaphore("bias"))
        pe_sem = ctx.enter_context(nc.semaphore("pe"))
        dve_sem = ctx.enter_context(nc.semaphore("dve"))
        act_sem = ctx.enter_context(nc.semaphore("act"))
        store_sem = ctx.enter_context(nc.semaphore("store"))

        block = ctx.enter_context(nc.Block())

        @block.sync
        def _(sync):
            sync.dma_start(out=Bt[:], in_=biasd[:]).then_inc(bias_sem, 16)
            for i, (c0, c1) in enumerate(zip(chunks[:-1], chunks[1:])):
                sync.dma_start(out=W[:, c0:c1], in_=pack[:, c0:c1]).then_inc(
                    csem[i], 16
                )
            sync.wait_ge(dve_sem, 15)
            sync.dma_start(out=out_d[0:1, :], in_=price[:]).then_inc(store_sem, 16)
            sync.wait_ge(act_sem, 1)
            sync.dma_start(out=out_d[1:2, :], in_=dirn[:]).then_inc(store_sem, 16)
            sync.wait_ge(store_sem, 32)

        @block.tensor
        def _(tensor):
            def mm(out, lhsT, rhs, start, stop, inc=False):
                m = nc.tensor.matmul(out, lhsT, rhs, start=start, stop=stop)
                if inc:
                    m.then_inc(pe_sem, 1)

            tensor.wait_ge(csem[0], 16)
            # input layer (K=64 quadrants, xT duplicated in both row halves)
            for m in range(2):
                r = slice(64 * m, 64 * (m + 1))
                mm(A[m][:], W[r, OFF_WIN : OFF_WIN + 128], W[r, 0:BPC],
                   True, True, inc=True)
            hh = h0
            for l in range(3):
                tensor.wait_ge(csem[1 + l], 16)
                tensor.wait_ge(dve_sem, 2 * (l + 1))
                out = (g1, g2, g3)[l]
                for m in range(2):
                    ms = slice(m * 128, (m + 1) * 128)
                    bank = A[(2 + 2 * l + m) % 4]
                    c0 = OFF_GAT + (2 * l) * 256
                    c1 = OFF_GAT + (2 * l + 1) * 256
                    mm(bank[:], W[:, c0 + m * 128 : c0 + m * 128 + 128], hh[0][:],
                       True, False)
                    mm(bank[:], W[:, c1 + m * 128 : c1 + m * 128 + 128], hh[1][:],
                       False, True, inc=True)
                hh = out
            tensor.wait_ge(csem[4], 16)
            tensor.wait_ge(dve_sem, 8)
            for m in range(2):
                bank = A[m]  # groups 8,9
                r = slice(64 * m, 64 * (m + 1))
                mm(bank[:], W[:, OFF_FUSE + m * 128 : OFF_FUSE + m * 128 + 128],
                   hh[0][:], True, False)
                mm(bank[:], W[:, OFF_FUSE + 256 + m * 128 : OFF_FUSE + 256 + m * 128 + 128],
                   hh[1][:], False, False)
                mm(bank[:], W[r, OFF_FUSE2 : OFF_FUSE2 + 128], W[r, BPC : 2 * BPC],
                   False, True, inc=True)
            tensor.wait_ge(csem[5], 16)
            tensor.wait_ge(dve_sem, 10)
            # p1 -> A2 (group 10), d1 -> A3 (group 11)
            mm(A[2][:], W[:, OFF_P1 : OFF_P1 + 128], fu[0][:], True, False)
            mm(A[2][:], W[:, OFF_P1 + 128 : OFF_P1 + 256], fu[1][:], False, True, inc=True)
            mm(A[3][:], W[:, OFF_D1 : OFF_D1 + 128], fu[0][:], True, False)
            mm(A[3][:], W[:, OFF_D1 + 128 : OFF_D1 + 256], fu[1][:], False, True, inc=True)
            tensor.wait_ge(dve_sem, 11)
            mm(Pp2[:], W[:, OFF_P2 : OFF_P2 + 64], a1p[:], True, True, inc=True)
            tensor.wait_ge(dve_sem, 12)
            mm(Pd2[:], W[:, OFF_D2 : OFF_D2 + 64], a1d[:], True, True, inc=True)
            tensor.wait_ge(dve_sem, 13)
            mm(Pp3[:], W[0:64, OFF_P3 : OFF_P3 + 1], a2p[:], True, True, inc=True)
            tensor.wait_ge(dve_sem, 14)
            mm(Pd3[:], W[0:64, OFF_D3 : OFF_D3 + 1], a2d[:], True, True, inc=True)

        @block.vector
        def _(vector):
            def rb(out, psum, j, pe_need, r1=128):
                vector.wait_ge(pe_sem, pe_need)
                nc.vector.tensor_scalar(
                    out, psum, Bt[0:r1, j : j + 1], 0.0, ADD, MAX
                ).then_inc(dve_sem, 1)

            vector.wait_ge(bias_sem, 16)
            rb(h0[0][:], A[0][:], 0, 1)
            rb(h0[1][:], A[1][:], 1, 2)
            for l, out in enumerate((g1, g2, g3)):
                for m in range(2):
                    rb(out[m][:], A[(2 + 2 * l + m) % 4][:], 2 + 2 * l + m,
                       3 + 2 * l + m)
            rb(fu[0][:], A[0][:], 8, 9)
            rb(fu[1][:], A[1][:], 9, 10)
            rb(a1p[:], A[2][:], 10, 11)
            rb(a1d[:], A[3][:], 11, 12)
            rb(a2p[:], Pp2[:], 12, 13, r1=64)
            rb(a2d[:], Pd2[:], 13, 14, r1=64)
            vector.wait_ge(pe_sem, 15)
            nc.vector.tensor_scalar(
                price[:], Pp3[:], Bt[0:1, 14:15], None, ADD
            ).then_inc(dve_sem, 1)

        @block.scalar
        def _(scalar):
            scalar.wait_ge(pe_sem, 16)
            nc.scalar.activation(
                dirn[:], Pd3[:], mybir.ActivationFunctionType.Sigmoid,
                bias=Bt[0:1, 15:16],
            ).then_inc(act_sem, 1)

    return nc


def _build_nc_tile():
    import concourse.bass as bass
    import concourse.mybir as mybir
    import concourse.tile as tile
    from concourse import bacc

    f32 = mybir.dt.float32
    f16 = mybir.dt.float16 if USE_F16 else mybir.dt.float32
    ADD = mybir.AluOpType.add
    MAX = mybir.AluOpType.max

    # Bacc (not raw Bass): its compile() splits multi-sem waits into event
    # semaphores, which TRN2's one-wait-per-instruction codegen requires.
    nc = bacc.Bacc("TRN2", target_bir_lowering=False, debug=False,
                   num_devices=N_CORES)
    pack = nc.declare_dram_parameter("pack", [128, COLS], f16, isOutput=False)
    biasd = nc.declare_dram_parameter("biasp", [128, 16], f32, isOutput=False)
    out_d = nc.declare_dram_parameter("out", [2, BPC], f32, isOutput=True)

    with tile.TileContext(nc) as tc:
        with (
            tc.tile_pool(name="w", bufs=1) as wp,
            tc.tile_pool(name="a", bufs=1) as ab,
            tc.tile_pool(name="ps", bufs=2, space=bass.MemorySpace.PSUM) as pp,
        ):
            W = wp.tile([128, COLS], f16, tag="W", name="W")
            Bt = wp.tile([128, 16], f32, tag="Bt", name="Bt")
            # chunked loads, ordered by consumption, so compute overlaps DMA
            chunks = [0, OFF_GAT, OFF_GAT + 512, OFF_GAT + 1024, OFF_FUSE,
                      OFF_P1, COLS]
            nc.sync.dma_start(out=Bt[:], in_=biasd[:])
            for c0, c1 in zip(chunks[:-1], chunks[1:]):
                nc.sync.dma_start(out=W[:, c0:c1], in_=pack[:, c0:c1])

            G = [W[:, OFF_GAT + i * 256 : OFF_GAT + (i + 1) * 256] for i in range(6)]
            F0 = W[:, OFF_FUSE : OFF_FUSE + 256]
            F1 = W[:, OFF_FUSE + 256 : OFF_FUSE + 512]
            F2 = W[:, OFF_FUSE2 : OFF_FUSE2 + 128]
            P1 = W[:, OFF_P1 : OFF_P1 + 256]
            D1 = W[:, OFF_D1 : OFF_D1 + 256]
            TAIL = W[:, OFF_P2:COLS]  # p2|d2|p3|d3

            def bias(j, r0=0, r1=128):
                return Bt[r0:r1, j : j + 1]

            def relu_bias(ps, j, shape, tag, r1=128):
                t = ab.tile(shape, f16, tag=tag, name=tag)
                nc.vector.tensor_scalar(t[:], ps[:], bias(j, 0, r1), 0.0, ADD, MAX)
                return t

            # input layer: h[m] = relu(W_in[m-tile] @ xT + b_in)
            h = []
            for m in range(2):
                r = slice(64 * m, 64 * (m + 1))
                ps = pp.tile([128, BPC], f32, tag="ps", name="ps")
                nc.tensor.matmul(ps[:], W[r, OFF_WIN : OFF_WIN + 128],
                                 W[r, 0:BPC], start=True, stop=True)
                h.append(relu_bias(ps, m, [128, BPC], f"h0_{m}"))

            # 3 collapsed GAT layers
            for l in range(3):
                nh = []
                for m in range(2):
                    ms = slice(m * 128, (m + 1) * 128)
                    ps = pp.tile([128, BPC], f32, tag="ps", name="ps")
                    nc.tensor.matmul(ps[:], G[2 * l][:, ms], h[0][:], start=True, stop=False)
                    nc.tensor.matmul(ps[:], G[2 * l + 1][:, ms], h[1][:], start=False, stop=True)
                    nh.append(relu_bias(ps, 2 + 2 * l + m, [128, BPC], f"h{l + 1}_{m}"))
                h = nh

            # fuse layer: concat([h, embT]) @ W_fuse.T
            f = []
            for m in range(2):
                ms = slice(m * 128, (m + 1) * 128)
                r = slice(64 * m, 64 * (m + 1))
                ps = pp.tile([128, BPC], f32, tag="ps", name="ps")
                nc.tensor.matmul(ps[:], F0[:, ms], h[0][:], start=True, stop=False)
                nc.tensor.matmul(ps[:], F1[:, ms], h[1][:], start=False, stop=False)
                nc.tensor.matmul(ps[:], F2[r, :], W[r, BPC : 2 * BPC], start=False, stop=True)
                f.append(relu_bias(ps, 8 + m, [128, BPC], f"f_{m}"))

            # heads: 256 -> 128 -> 64 -> 1
            def head(W1, c2, c3, j1, j2, j3, sigmoid):
                ps = pp.tile([128, BPC], f32, tag="ps", name="ps")
                nc.tensor.matmul(ps[:], W1[:, 0:128], f[0][:], start=True, stop=False)
                nc.tensor.matmul(ps[:], W1[:, 128:256], f[1][:], start=False, stop=True)
                a1 = relu_bias(ps, j1, [128, BPC], f"a1_{j1}")
                ps2 = pp.tile([64, BPC], f32, tag="ps2", name="ps2")
                nc.tensor.matmul(ps2[:], TAIL[:, c2 : c2 + 64], a1[:], start=True, stop=True)
                a2 = relu_bias(ps2, j2, [64, BPC], f"a2_{j2}", r1=64)
                ps3 = pp.tile([1, BPC], f32, tag="ps3", name="ps3")
                nc.tensor.matmul(ps3[:], TAIL[0:64, c3 : c3 + 1], a2[:], start=True, stop=True)
                resn = f"res_{j3}"
                res = ab.tile([1, BPC], f32, tag=resn, name=resn)
                if sigmoid:
                    nc.scalar.activation(
                        res[:], ps3[:],
                        mybir.ActivationFunctionType.Sigmoid, bias=bias(j3, 0, 1),
                    )
                else:
                    nc.vector.tensor_scalar(res[:], ps3[:], bias(j3, 0, 1), None, ADD)
                return res

            price = head(P1, 0, 128, 10, 12, 14, sigmoid=False)
            dirn = head(D1, 64, 129, 11, 13, 15, sigmoid=True)
            # two independent stores on different DMA engines so they overlap
            nc.sync.dma_start(out=out_d[0:1, :], in_=price[:])
            nc.gpsimd.dma_start(out=out_d[1:2, :], in_=dirn[:])

    nc.compile()
    return nc


def _pack_host(inputs):
    f32 = lambda k: np.ascontiguousarray(np.asarray(inputs[k], dtype=np.float32))
    W_in, b_in = f32("W_in"), f32("b_in")
    gat_W, gat_b = f32("gat_W"), f32("gat_b")
    W_fuse, b_fuse = f32("W_fuse"), f32("b_fuse")
    W_p1, b_p1 = f32("W_p1"), f32("b_p1")
    W_p2, b_p2 = f32("W_p2"), f32("b_p2")
    W_p3, b_p3 = f32("W_p3"), f32("b_p3")
    W_d1, b_d1 = f32("W_d1"), f32("b_d1")
    W_d2, b_d2 = f32("W_d2"), f32("b_d2")
    W_d3, b_d3 = f32("W_d3"), f32("b_d3")

    bias = np.zeros((128, 16), np.float32)
    bias[:, 0], bias[:, 1] = b_in[:128], b_in[128:]
    for l in range(3):
        for m in range(2):
            bias[:, 2 + 2 * l + m] = gat_b[l, 128 * m : 128 * (m + 1)]
    bias[:, 8], bias[:, 9] = b_fuse[:128], b_fuse[128:]
    bias[:, 10], bias[:, 11] = b_p1, b_d1
    bias[:64, 12], bias[:64, 13] = b_p2, b_d2
    bias[0, 14], bias[0, 15] = b_p3[0], b_d3[0]

    np16 = np.float16 if USE_F16 else np.float32
    pk = np.zeros((128, COLS), np16)
    WinT = W_in.T.astype(np16)  # [64, 256]
    pk[0:64, OFF_WIN : OFF_WIN + 128] = WinT[:, 0:128]
    pk[64:128, OFF_WIN : OFF_WIN + 128] = WinT[:, 128:256]
    for l in range(3):
        GT = gat_W[l].T.astype(np16)  # [256, 256]
        for k in range(2):
            c = OFF_GAT + (2 * l + k) * 256
            pk[:, c : c + 256] = GT[128 * k : 128 * (k + 1), :]
    FT = W_fuse.T.astype(np16)  # [320, 256]
    pk[:, OFF_FUSE : OFF_FUSE + 256] = FT[0:128]
    pk[:, OFF_FUSE + 256 : OFF_FUSE + 512] = FT[128:256]
    pk[0:64, OFF_FUSE2 : OFF_FUSE2 + 128] = FT[256:320, 0:128]
    pk[64:128, OFF_FUSE2 : OFF_FUSE2 + 128] = FT[256:320, 128:256]
    for W1, off in ((W_p1, OFF_P1), (W_d1, OFF_D1)):
        T = W1.T.astype(np16)  # [256, 128]
        pk[:, off : off + 128] = T[0:128]
        pk[:, off + 128 : off + 256] = T[128:256]
    pk[:, OFF_P2 : OFF_P2 + 64] = W_p2.T.astype(np16)
    pk[:, OFF_D2 : OFF_D2 + 64] = W_d2.T.astype(np16)
    pk[0:64, OFF_P3] = W_p3[0].astype(np16)
    pk[0:64, OFF_D3] = W_d3[0].astype(np16)
    return pk, bias


def kernel(**inputs):
    if "nc" not in _CACHE:
        _CACHE["nc"] = _build_nc()
    nc = _CACHE["nc"]
    from concourse.bass_utils import run_bass_kernel_spmd

    x = np.asarray(inputs["x"], dtype=np.float32)
    ci = np.asarray(inputs["company_indices"]).astype(np.int64)
    emb = np.asarray(inputs["emb"], dtype=np.float32)
    comp_emb = emb[ci]  # [B, 64]

    base, bias = _pack_host(inputs)
    in_maps = []
    for c in range(N_CORES):
        pk = base.copy()
        rows = slice(c * BPC, (c + 1) * BPC)
        xT = x[rows].T.astype(base.dtype)  # [64, BPC]
        eT = comp_emb[rows].T.astype(base.dtype)
        pk[0:64, 0:BPC] = xT
        pk[64:128, 0:BPC] = xT
        pk[0:64, BPC : 2 * BPC] = eT
        pk[64:128, BPC : 2 * BPC] = eT
        in_maps.append({"pack": pk, "biasp": bias})

    res = run_bass_kernel_spmd(nc, in_maps, list(range(N_CORES)))
    outs = res.results
    price = np.concatenate([outs[c]["out"][0] for c in range(N_CORES)]).astype(np.float32)
    direction = np.concatenate([outs[c]["out"][1] for c in range(N_CORES)]).astype(np.float32)
    return price, direction


# revision 13
# speedup vs baseline: 1.0595x; 1.0595x over previous
"""Trainium2 Bass kernel for nn_EnhancedFinancialGAT.

Mathematical collapse: the reference broadcasts each batch item's feature
vector to all N=2000 graph nodes, so every node starts identical. A GAT
layer on identical node features returns, for every node, the attention-
weighted average of identical projected vectors -- and per-dst softmax
weights sum to exactly 1 in f32 (denom + 1e-16 == denom), so each layer
reduces to relu(h @ W.T + b). Every node stays identical through all 3
layers, and the company-node gather picks that shared vector. The whole
model is therefore an MLP:

  h = relu(x @ W_in.T + b_in)
  h = relu(h @ gat_W[l].T + gat_b[l])   for l in 0..2
  fused = relu(concat([h, emb[company_indices]]) @ W_fuse.T + b_fuse)
  price = W_p3 @ relu(W_p2 @ relu(W_p1 @ fused + b_p1) + b_p2) + b_p3
  direction = sigmoid(same with d-weights)

Verified numerically: collapsed-vs-full relative error ~2e-7 (pure f32
rounding noise of the softmax-weighted sums).

Sharding: data-parallel over batch (64 rows -> 8 rows/core). Weights are
replicated, pre-transposed on host into one packed fp16 [128, COLS] DRAM
tensor per core (activations-transposed layout [feature, batch], so no
on-device transposes are needed). fp16 matmul inputs with fp32 PSUM
accumulation and fp32 biases keep the end-to-end error ~1e-4.
"""

import numpy as np

USE_F16 = False  # fp32 is exact (6e-7); fp16 lands ~1e-3 (PE computes 16-bit at bf16 precision)

B = 64
N_CORES = 8
BPC = B // N_CORES  # batch rows per core

# -------- packed fp16 [128, COLS] layout (column offsets) --------
OFF_ACTS = 0                       # 16 cols: xT 0:8, embT 8:16 (rows 0:64, dup to 64:128)
OFF_WIN = 16                       # 128 cols: rows 0:64 -> M[0:128], rows 64:128 -> M[128:256]
OFF_GAT = 144                      # 6 blocks of 256 cols: block (l,k) at (2l+k)*256
OFF_FUSE = OFF_GAT + 6 * 256       # k0 [128,256], k1 [128,256]
OFF_FUSE2 = OFF_FUSE + 512         # k2 row-split, 128 cols
OFF_P1 = OFF_FUSE2 + 128           # k0 cols 0:128, k1 cols 128:256
OFF_D1 = OFF_P1 + 256
OFF_P2 = OFF_D1 + 256              # [128, 64]
OFF_D2 = OFF_P2 + 64
OFF_P3 = OFF_D2 + 64               # rows 0:64, 1 col
OFF_D3 = OFF_P3 + 1
COLS = OFF_D3 + 1                  # 2962

# fp32 bias tensor [128, 16] column map:
#  0,1: b_in | 2..7: gat_b (l,m) | 8,9: b_fuse | 10: b_p1 | 11: b_d1
#  12: b_p2 (rows 0:64) | 13: b_d2 | 14 row0: b_p3 | 15 row0: b_d3

_CACHE = {}

RAW = True  # raw-Bass build (manual semaphores, no Tile preamble/tail barriers)


def _build_nc():
    return _build_nc_raw() if RAW else _build_nc_tile()


def _build_nc_raw():
    """Raw Bass: explicit per-engine programs + semaphores, no Block.

    Everything lives in the single entry basic block: no branches (so no
    IRAM I$-miss stalls at block boundaries), no Tile/Block exit
    all-engine barriers. Completion: output DMAs inc store_sem; gpsimd is
    the sole final waiter and clears our semaphores so the NEFF can be
    re-executed. PSUM: 4 rotating [128,8] banks for the wide groups (WAR
    is covered by the RAW waits since DVE is in-order) + 4 dedicated
    banks for the head tails = exactly 8.
    """
    from contextlib import ExitStack

    import concourse.bass as bass
    import concourse.mybir as mybir

    f32 = mybir.dt.float32
    f16 = mybir.dt.float16 if USE_F16 else mybir.dt.float32
    ADD = mybir.AluOpType.add
    MAX = mybir.AluOpType.max

    nc = bass.Bass("TRN2", debug=False, num_devices=N_CORES)
    pack = nc.declare_dram_parameter("pack", [128, COLS], f16, isOutput=False)
    biasd = nc.declare_dram_parameter("biasp", [128, 16], f32, isOutput=False)
    out_d = nc.declare_dram_parameter("out", [2, BPC], f32, isOutput=True)

    chunks = [0, OFF_GAT, OFF_GAT + 512, OFF_GAT + 1024, OFF_FUSE, OFF_P1, COLS]

    ctx = ExitStack()
    with ctx:
        sb = lambda nm, shape, dt: ctx.enter_context(nc.sbuf_tensor(nm, shape, dt))
        psb = lambda nm, shape: ctx.enter_context(nc.psum_tensor(nm, shape, f32))
        W = sb("W", [128, COLS], f16)
        Bt = sb("Bt", [128, 16], f32)
        h0 = [sb(f"h0_{i}", [128, BPC], f16) for i in range(2)]
        g1 = [sb(f"g1_{i}", [128, BPC], f16) for i in range(2)]
        g2 = [sb(f"g2_{i}", [128, BPC], f16) for i in range(2)]
        g3 = [sb(f"g3_{i}", [128, BPC], f16) for i in range(2)]
        fu = [sb(f"fu_{i}", [128, BPC], f16) for i in range(2)]
        a1p = sb("a1p", [128, BPC], f16)
        a1d = sb("a1d", [128, BPC], f16)
        a2p = sb("a2p", [64, BPC], f16)
        a2d = sb("a2d", [64, BPC], f16)
        price = sb("price", [1, BPC], f32)
        dirn = sb("dirn", [1, BPC], f32)
        scratch = sb("scratch", [1, BPC], f32)
        A = [psb(f"A{i}", [128, BPC]) for i in range(4)]
        Pp2 = psb("Pp2", [64, BPC])
        Pd2 = psb("Pd2", [64, BPC])
        Pp3 = psb("Pp3", [1, BPC])
        Pd3 = psb("Pd3", [1, BPC])

        csem = [ctx.enter_context(nc.semaphore(f"c{i}")) for i in range(6)]
        bias_sem = ctx.enter_context(nc.semaphore("bias"))
        pe_sem = ctx.enter_context(nc.semaphore("pe"))
        dve_sem = ctx.enter_context(nc.semaphore("dve"))
        store_sem = ctx.enter_context(nc.semaphore("store"))
        all_sems = csem + [bias_sem, pe_sem, dve_sem, store_sem]

        # ---- loads: c0 first (input layer), HWDGE on sync for c0-c3,
        # SWDGE on gpsimd for the late chunks c4, c5 (parallel issue) ----
        def load(eng, i):
            c0, c1 = chunks[i], chunks[i + 1]
            eng.dma_start(out=W[:, c0:c1], in_=pack[:, c0:c1]).then_inc(csem[i], 16)

        load(nc.sync, 0)
        nc.sync.dma_start(out=Bt[:], in_=biasd[:]).then_inc(bias_sem, 16)
        for i in (1, 2, 3):
            load(nc.sync, i)
        for i in (4, 5):
            load(nc.gpsimd, i)

        # ---- ACT: preload the sigmoid table set early (off critical path) --
        nc.scalar.activation(
            scratch[:], scratch[:], mybir.ActivationFunctionType.Sigmoid
        )

        # ---- PE program -------------------------------------------------
        pe = nc.tensor
        peng = nc.engines[mybir.EngineType.PE]

        def mm(out, lhsT, rhs, start, stop, inc=False):
            m = pe.matmul(out, lhsT, rhs, start=start, stop=stop)
            if inc:
                m.then_inc(pe_sem, 1)

        peng.wait_ge(csem[0], 16)
        for m in range(2):
            r = slice(64 * m, 64 * (m + 1))
            mm(A[m][:], W[r, OFF_WIN : OFF_WIN + 128], W[r, 0:BPC],
               True, True, inc=True)
        hh = h0
        for l in range(3):
            out = (g1, g2, g3)[l]
            c0 = OFF_GAT + (2 * l) * 256
            c1 = OFF_GAT + (2 * l + 1) * 256
            bank = [A[(2 + 2 * l) % 4], A[(3 + 2 * l) % 4]]
            peng.wait_ge(csem[1 + l], 16)
            peng.wait_ge(dve_sem, 2 * l + 1)          # hh[0] ready
            for m in range(2):
                mm(bank[m][:], W[:, c0 + m * 128 : c0 + m * 128 + 128],
                   hh[0][:], True, False)
            peng.wait_ge(dve_sem, 2 * l + 2)          # hh[1] ready
            for m in range(2):
                mm(bank[m][:], W[:, c1 + m * 128 : c1 + m * 128 + 128],
                   hh[1][:], False, True, inc=True)
            hh = out
        peng.wait_ge(csem[4], 16)
        peng.wait_ge(dve_sem, 7)
        for m in range(2):
            mm(A[m][:], W[:, OFF_FUSE + m * 128 : OFF_FUSE + m * 128 + 128],
               hh[0][:], True, False)
        peng.wait_ge(dve_sem, 8)
        for m in range(2):
            mm(A[m][:], W[:, OFF_FUSE + 256 + m * 128 : OFF_FUSE + 256 + m * 128 + 128],
               hh[1][:], False, False)
        for m in range(2):
            r = slice(64 * m, 64 * (m + 1))
            mm(A[m][:], W[r, OFF_FUSE2 : OFF_FUSE2 + 128], W[r, BPC : 2 * BPC],
               False, True, inc=True)
        peng.wait_ge(csem[5], 16)
        peng.wait_ge(dve_sem, 9)                      # fu[0]
        mm(A[2][:], W[:, OFF_P1 : OFF_P1 + 128], fu[0][:], True, False)
        mm(A[3][:], W[:, OFF_D1 : OFF_D1 + 128], fu[0][:], True, False)
        peng.wait_ge(dve_sem, 10)                     # fu[1]
        mm(A[2][:], W[:, OFF_P1 + 128 : OFF_P1 + 256], fu[1][:], False, True, inc=True)
        mm(A[3][:], W[:, OFF_D1 + 128 : OFF_D1 + 256], fu[1][:], False, True, inc=True)
        peng.wait_ge(dve_sem, 11)
        mm(Pp2[:], W[:, OFF_P2 : OFF_P2 + 64], a1p[:], True, True, inc=True)
        peng.wait_ge(dve_sem, 12)
        mm(Pd2[:], W[:, OFF_D2 : OFF_D2 + 64], a1d[:], True, True, inc=True)
        peng.wait_ge(dve_sem, 13)
        mm(Pp3[:], W[0:64, OFF_P3 : OFF_P3 + 1], a2p[:], True, True, inc=True)
        peng.wait_ge(dve_sem, 14)
        mm(Pd3[:], W[0:64, OFF_D3 : OFF_D3 + 1], a2d[:], True, True, inc=True)

        # ---- DVE program ------------------------------------------------
        veng = nc.engines[mybir.EngineType.DVE]

        def rb(out, psum, j, pe_need, r1=128):
            veng.wait_ge(pe_sem, pe_need)
            nc.vector.tensor_scalar(
                out, psum, Bt[0:r1, j : j + 1], 0.0, ADD, MAX
            ).then_inc(dve_sem, 1)

        veng.wait_ge(bias_sem, 16)
        rb(h0[0][:], A[0][:], 0, 1)
        rb(h0[1][:], A[1][:], 1, 2)
        for l, out in enumerate((g1, g2, g3)):
            for m in range(2):
                rb(out[m][:], A[(2 + 2 * l + m) % 4][:], 2 + 2 * l + m,
                   3 + 2 * l + m)
        rb(fu[0][:], A[0][:], 8, 9)
        rb(fu[1][:], A[1][:], 9, 10)
        rb(a1p[:], A[2][:], 10, 11)
        rb(a1d[:], A[3][:], 11, 12)
        rb(a2p[:], Pp2[:], 12, 13, r1=64)
        rb(a2d[:], Pd2[:], 13, 14, r1=64)
        veng.wait_ge(pe_sem, 15)
        nc.vector.tensor_scalar(
            price[:], Pp3[:], Bt[0:1, 14:15], None, ADD
        ).then_inc(dve_sem, 1)

        # ---- ACT: real sigmoid + dir store (scalar is HWDGE-capable) ----
        aeng = nc.engines[mybir.EngineType.Activation]
        aeng.wait_ge(pe_sem, 16)
        nc.scalar.activation(
            dirn[:], Pd3[:], mybir.ActivationFunctionType.Sigmoid,
            bias=Bt[0:1, 15:16],
        )
        nc.scalar.dma_start(out=out_d[1:2, :], in_=dirn[:]).then_inc(store_sem, 16)

        # ---- sync: price store ------------------------------------------
        seng = nc.engines[mybir.EngineType.SP]
        seng.wait_ge(dve_sem, 15)
        nc.sync.dma_start(out=out_d[0:1, :], in_=price[:]).then_inc(store_sem, 16)

        # ---- gpsimd: sole final waiter; clear sems for re-execution -----
        geng = nc.engines[mybir.EngineType.Pool]
        geng.wait_ge(store_sem, 32)
        nums = sorted(s.num for s in all_sems)
        from concourse._compat import not_none
        lo = nums[0]
        hi = nums[-1]
        assert nums == list(range(lo, hi + 1)), nums
        nc.gpsimd.dma_reset(range(lo, hi + 1))
        nc.gpsimd.sem_clear(range(lo, hi + 1))

    return nc


def _build_nc_tile():
    import concourse.bass as bass
    import concourse.mybir as mybir
    import concourse.tile as tile
    from concourse import bacc

    f32 = mybir.dt.float32
    f16 = mybir.dt.float16 if USE_F16 else mybir.dt.float32
    ADD = mybir.AluOpType.add
    MAX = mybir.AluOpType.max

    # Bacc (not raw Bass): its compile() splits multi-sem waits into event
    # semaphores, which TRN2's one-wait-per-instruction codegen requires.
    nc = bacc.Bacc("TRN2", target_bir_lowering=False, debug=False,
                   num_devices=N_CORES)
    pack = nc.declare_dram_parameter("pack", [128, COLS], f16, isOutput=False)
    biasd = nc.declare_dram_parameter("biasp", [128, 16], f32, isOutput=False)
    out_d = nc.declare_dram_parameter("out", [2, BPC], f32, isOutput=True)

    with tile.TileContext(nc) as tc:
        with (
            tc.tile_pool(name="w", bufs=1) as wp,
            tc.tile_pool(name="a", bufs=1) as ab,
            tc.tile_pool(name="ps", bufs=2, space=bass.MemorySpace.PSUM) as pp,
        ):
            W = wp.tile([128, COLS], f16, tag="W", name="W")
            Bt = wp.tile([128, 16], f32, tag="Bt", name="Bt")
            # chunked loads, ordered by consumption, so compute overlaps DMA
            chunks = [0, OFF_GAT, OFF_GAT + 512, OFF_GAT + 1024, OFF_FUSE,
                      OFF_P1, COLS]
            nc.sync.dma_start(out=Bt[:], in_=biasd[:])
            for c0, c1 in zip(chunks[:-1], chunks[1:]):
                nc.sync.dma_start(out=W[:, c0:c1], in_=pack[:, c0:c1])

            G = [W[:, OFF_GAT + i * 256 : OFF_GAT + (i + 1) * 256] for i in range(6)]
            F0 = W[:, OFF_FUSE : OFF_FUSE + 256]
            F1 = W[:, OFF_FUSE + 256 : OFF_FUSE + 512]
            F2 = W[:, OFF_FUSE2 : OFF_FUSE2 + 128]
            P1 = W[:, OFF_P1 : OFF_P1 + 256]
            D1 = W[:, OFF_D1 : OFF_D1 + 256]
            TAIL = W[:, OFF_P2:COLS]  # p2|d2|p3|d3

            def bias(j, r0=0, r1=128):
                return Bt[r0:r1, j : j + 1]

            def relu_bias(ps, j, shape, tag, r1=128):
                t = ab.tile(shape, f16, tag=tag, name=tag)
                nc.vector.tensor_scalar(t[:], ps[:], bias(j, 0, r1), 0.0, ADD, MAX)
                return t

            # input layer: h[m] = relu(W_in[m-tile] @ xT + b_in)
            h = []
            for m in range(2):
                r = slice(64 * m, 64 * (m + 1))
                ps = pp.tile([128, BPC], f32, tag="ps", name="ps")
                nc.tensor.matmul(ps[:], W[r, OFF_WIN : OFF_WIN + 128],
                                 W[r, 0:BPC], start=True, stop=True)
                h.append(relu_bias(ps, m, [128, BPC], f"h0_{m}"))

            # 3 collapsed GAT layers
            for l in range(3):
                nh = []
                for m in range(2):
                    ms = slice(m * 128, (m + 1) * 128)
                    ps = pp.tile([128, BPC], f32, tag="ps", name="ps")
                    nc.tensor.matmul(ps[:], G[2 * l][:, ms], h[0][:], start=True, stop=False)
                    nc.tensor.matmul(ps[:], G[2 * l + 1][:, ms], h[1][:], start=False, stop=True)
                    nh.append(relu_bias(ps, 2 + 2 * l + m, [128, BPC], f"h{l + 1}_{m}"))
                h = nh

            # fuse layer: concat([h, embT]) @ W_fuse.T
            f = []
            for m in range(2):
                ms = slice(m * 128, (m + 1) * 128)
                r = slice(64 * m, 64 * (m + 1))
                ps = pp.tile([128, BPC], f32, tag="ps", name="ps")
                nc.tensor.matmul(ps[:], F0[:, ms], h[0][:], start=True, stop=False)
                nc.tensor.matmul(ps[:], F1[:, ms], h[1][:], start=False, stop=False)
                nc.tensor.matmul(ps[:], F2[r, :], W[r, BPC : 2 * BPC], start=False, stop=True)
                f.append(relu_bias(ps, 8 + m, [128, BPC], f"f_{m}"))

            # heads: 256 -> 128 -> 64 -> 1
            def head(W1, c2, c3, j1, j2, j3, sigmoid):
                ps = pp.tile([128, BPC], f32, tag="ps", name="ps")
                nc.tensor.matmul(ps[:], W1[:, 0:128], f[0][:], start=True, stop=False)
                nc.tensor.matmul(ps[:], W1[:, 128:256], f[1][:], start=False, stop=True)
                a1 = relu_bias(ps, j1, [128, BPC], f"a1_{j1}")
                ps2 = pp.tile([64, BPC], f32, tag="ps2", name="ps2")
                nc.tensor.matmul(ps2[:], TAIL[:, c2 : c2 + 64], a1[:], start=True, stop=True)
                a2 = relu_bias(ps2, j2, [64, BPC], f"a2_{j2}", r1=64)
                ps3 = pp.tile([1, BPC], f32, tag="ps3", name="ps3")
                nc.tensor.matmul(ps3[:], TAIL[0:64, c3 : c3 + 1], a2[:], start=True, stop=True)
                resn = f"res_{j3}"
                res = ab.tile([1, BPC], f32, tag=resn, name=resn)
                if sigmoid:
                    nc.scalar.activation(
                        res[:], ps3[:],
                        mybir.ActivationFunctionType.Sigmoid, bias=bias(j3, 0, 1),
                    )
                else:
                    nc.vector.tensor_scalar(res[:], ps3[:], bias(j3, 0, 1), None, ADD)
                return res

            price = head(P1, 0, 128, 10, 12, 14, sigmoid=False)
            dirn = head(D1, 64, 129, 11, 13, 15, sigmoid=True)
            # two independent stores on different DMA engines so they overlap
            nc.sync.dma_start(out=out_d[0:1, :], in_=price[:])
            nc.gpsimd.dma_start(out=out_d[1:2, :], in_=dirn[:])

    nc.compile()
    return nc


def _pack_host(inputs):
    f32 = lambda k: np.ascontiguousarray(np.asarray(inputs[k], dtype=np.float32))
    W_in, b_in = f32("W_in"), f32("b_in")
    gat_W, gat_b = f32("gat_W"), f32("gat_b")
    W_fuse, b_fuse = f32("W_fuse"), f32("b_fuse")
    W_p1, b_p1 = f32("W_p1"), f32("b_p1")
    W_p2, b_p2 = f32("W_p2"), f32("b_p2")
    W_p3, b_p3 = f32("W_p3"), f32("b_p3")
    W_d1, b_d1 = f32("W_d1"), f32("b_d1")
    W_d2, b_d2 = f32("W_d2"), f32("b_d2")
    W_d3, b_d3 = f32("W_d3"), f32("b_d3")

    bias = np.zeros((128, 16), np.float32)
    bias[:, 0], bias[:, 1] = b_in[:128], b_in[128:]
    for l in range(3):
        for m in range(2):
            bias[:, 2 + 2 * l + m] = gat_b[l, 128 * m : 128 * (m + 1)]
    bias[:, 8], bias[:, 9] = b_fuse[:128], b_fuse[128:]
    bias[:, 10], bias[:, 11] = b_p1, b_d1
    bias[:64, 12], bias[:64, 13] = b_p2, b_d2
    bias[0, 14], bias[0, 15] = b_p3[0], b_d3[0]

    np16 = np.float16 if USE_F16 else np.float32
    pk = np.zeros((128, COLS), np16)
    WinT = W_in.T.astype(np16)  # [64, 256]
    pk[0:64, OFF_WIN : OFF_WIN + 128] = WinT[:, 0:128]
    pk[64:128, OFF_WIN : OFF_WIN + 128] = WinT[:, 128:256]
    for l in range(3):
        GT = gat_W[l].T.astype(np16)  # [256, 256]
        for k in range(2):
            c = OFF_GAT + (2 * l + k) * 256
            pk[:, c : c + 256] = GT[128 * k : 128 * (k + 1), :]
    FT = W_fuse.T.astype(np16)  # [320, 256]
    pk[:, OFF_FUSE : OFF_FUSE + 256] = FT[0:128]
    pk[:, OFF_FUSE + 256 : OFF_FUSE + 512] = FT[128:256]
    pk[0:64, OFF_FUSE2 : OFF_FUSE2 + 128] = FT[256:320, 0:128]
    pk[64:128, OFF_FUSE2 : OFF_FUSE2 + 128] = FT[256:320, 128:256]
    for W1, off in ((W_p1, OFF_P1), (W_d1, OFF_D1)):
        T = W1.T.astype(np16)  # [256, 128]
        pk[:, off : off + 128] = T[0:128]
        pk[:, off + 128 : off + 256] = T[128:256]
    pk[:, OFF_P2 : OFF_P2 + 64] = W_p2.T.astype(np16)
    pk[:, OFF_D2 : OFF_D2 + 64] = W_d2.T.astype(np16)
    pk[0:64, OFF_P3] = W_p3[0].astype(np16)
    pk[0:64, OFF_D3] = W_d3[0].astype(np16)
    return pk, bias


def kernel(**inputs):
    if "nc" not in _CACHE:
        _CACHE["nc"] = _build_nc()
    nc = _CACHE["nc"]
    from concourse.bass_utils import run_bass_kernel_spmd

    x = np.asarray(inputs["x"], dtype=np.float32)
    ci = np.asarray(inputs["company_indices"]).astype(np.int64)
    emb = np.asarray(inputs["emb"], dtype=np.float32)
    comp_emb = emb[ci]  # [B, 64]

    base, bias = _pack_host(inputs)
    in_maps = []
    for c in range(N_CORES):
        pk = base.copy()
        rows = slice(c * BPC, (c + 1) * BPC)
        xT = x[rows].T.astype(base.dtype)  # [64, BPC]
        eT = comp_emb[rows].T.astype(base.dtype)
        pk[0:64, 0:BPC] = xT
        pk[64:128, 0:BPC] = xT
        pk[0:64, BPC : 2 * BPC] = eT
        pk[64:128, BPC : 2 * BPC] = eT
        in_maps.append({"pack": pk, "biasp": bias})

    res = run_bass_kernel_spmd(nc, in_maps, list(range(N_CORES)))
    outs = res.results
    price = np.concatenate([outs[c]["out"][0] for c in range(N_CORES)]).astype(np.float32)
    direction = np.concatenate([outs[c]["out"][1] for c in range(N_CORES)]).astype(np.float32)
    return price, direction


# revision 15
# speedup vs baseline: 1.0701x; 1.0100x over previous
"""Trainium2 Bass kernel for nn_EnhancedFinancialGAT.

Mathematical collapse: the reference broadcasts each batch item's feature
vector to all N=2000 graph nodes, so every node starts identical. A GAT
layer on identical node features returns, for every node, the attention-
weighted average of identical projected vectors -- and per-dst softmax
weights sum to exactly 1 in f32 (denom + 1e-16 == denom), so each layer
reduces to relu(h @ W.T + b). Every node stays identical through all 3
layers, and the company-node gather picks that shared vector. The whole
model is therefore an MLP:

  h = relu(x @ W_in.T + b_in)
  h = relu(h @ gat_W[l].T + gat_b[l])   for l in 0..2
  fused = relu(concat([h, emb[company_indices]]) @ W_fuse.T + b_fuse)
  price = W_p3 @ relu(W_p2 @ relu(W_p1 @ fused + b_p1) + b_p2) + b_p3
  direction = sigmoid(same with d-weights)

Verified numerically: collapsed-vs-full relative error ~2e-7 (pure f32
rounding noise of the softmax-weighted sums).

Sharding: data-parallel over batch (64 rows -> 8 rows/core). Weights are
replicated, pre-transposed on host into one packed fp16 [128, COLS] DRAM
tensor per core (activations-transposed layout [feature, batch], so no
on-device transposes are needed). fp16 matmul inputs with fp32 PSUM
accumulation and fp32 biases keep the end-to-end error ~1e-4.
"""

import numpy as np

USE_F16 = False  # fp32 is exact (6e-7); fp16 lands ~1e-3 (PE computes 16-bit at bf16 precision)

B = 64
N_CORES = 8
BPC = B // N_CORES  # batch rows per core

# -------- packed fp16 [128, COLS] layout (column offsets) --------
OFF_ACTS = 0                       # 16 cols: xT 0:8, embT 8:16 (rows 0:64, dup to 64:128)
OFF_WIN = 16                       # 128 cols: rows 0:64 -> M[0:128], rows 64:128 -> M[128:256]
OFF_GAT = 144                      # 6 blocks of 256 cols: block (l,k) at (2l+k)*256
OFF_FUSE = OFF_GAT + 6 * 256       # k0 [128,256], k1 [128,256]
OFF_FUSE2 = OFF_FUSE + 512         # k2 row-split, 128 cols
OFF_P1 = OFF_FUSE2 + 128           # k0 cols 0:128, k1 cols 128:256
OFF_D1 = OFF_P1 + 256
OFF_P2 = OFF_D1 + 256              # [128, 64]
OFF_D2 = OFF_P2 + 64
OFF_P3 = OFF_D2 + 64               # rows 0:64, 1 col
OFF_D3 = OFF_P3 + 1
COLS = OFF_D3 + 1                  # 2962

# fp32 bias tensor [128, 16] column map:
#  0,1: b_in | 2..7: gat_b (l,m) | 8,9: b_fuse | 10: b_p1 | 11: b_d1
#  12: b_p2 (rows 0:64) | 13: b_d2 | 14 row0: b_p3 | 15 row0: b_d3

_CACHE = {}

RAW = True  # raw-Bass build (manual semaphores, no Tile preamble/tail barriers)


def _build_nc():
    return _build_nc_raw() if RAW else _build_nc_tile()


def _build_nc_raw():
    """Raw Bass: explicit per-engine programs + semaphores, no Block.

    Everything lives in the single entry basic block: no branches (so no
    IRAM I$-miss stalls at block boundaries), no Tile/Block exit
    all-engine barriers. Completion: output DMAs inc store_sem; gpsimd is
    the sole final waiter and clears our semaphores so the NEFF can be
    re-executed. PSUM: 4 rotating [128,8] banks for the wide groups (WAR
    is covered by the RAW waits since DVE is in-order) + 4 dedicated
    banks for the head tails = exactly 8.
    """
    from contextlib import ExitStack

    import concourse.bass as bass
    import concourse.mybir as mybir

    f32 = mybir.dt.float32
    f16 = mybir.dt.float16 if USE_F16 else mybir.dt.float32
    ADD = mybir.AluOpType.add
    MAX = mybir.AluOpType.max

    nc = bass.Bass("TRN2", debug=False, num_devices=N_CORES)
    chunks = [0, OFF_GAT, OFF_GAT + 512, OFF_GAT + 1024, OFF_FUSE, OFF_P1, COLS]
    # one contiguous DRAM tensor per chunk: strided slices of a single wide
    # tensor DMA at ~40GB/s; contiguous blocks run near line rate
    packs = [
        nc.declare_dram_parameter(f"pack{i}", [128, c1 - c0], f16, isOutput=False)
        for i, (c0, c1) in enumerate(zip(chunks[:-1], chunks[1:]))
    ]
    biasd = nc.declare_dram_parameter("biasp", [128, 16], f32, isOutput=False)
    out_d = nc.declare_dram_parameter("out", [2, BPC], f32, isOutput=True)

    ctx = ExitStack()
    with ctx:
        sb = lambda nm, shape, dt: ctx.enter_context(nc.sbuf_tensor(nm, shape, dt))
        psb = lambda nm, shape: ctx.enter_context(nc.psum_tensor(nm, shape, f32))
        W = sb("W", [128, COLS], f16)
        Bt = sb("Bt", [128, 16], f32)
        h0 = [sb(f"h0_{i}", [128, BPC], f16) for i in range(2)]
        g1 = [sb(f"g1_{i}", [128, BPC], f16) for i in range(2)]
        g2 = [sb(f"g2_{i}", [128, BPC], f16) for i in range(2)]
        g3 = [sb(f"g3_{i}", [128, BPC], f16) for i in range(2)]
        fu = [sb(f"fu_{i}", [128, BPC], f16) for i in range(2)]
        a1p = sb("a1p", [128, BPC], f16)
        a1d = sb("a1d", [128, BPC], f16)
        a2p = sb("a2p", [64, BPC], f16)
        a2d = sb("a2d", [64, BPC], f16)
        price = sb("price", [1, BPC], f32)
        dirn = sb("dirn", [1, BPC], f32)
        scratch = sb("scratch", [1, BPC], f32)
        A = [psb(f"A{i}", [128, BPC]) for i in range(4)]
        Pp2 = psb("Pp2", [64, BPC])
        Pd2 = psb("Pd2", [64, BPC])
        Pp3 = psb("Pp3", [1, BPC])
        Pd3 = psb("Pd3", [1, BPC])

        csem = [ctx.enter_context(nc.semaphore(f"c{i}")) for i in range(6)]
        bias_sem = ctx.enter_context(nc.semaphore("bias"))
        pe_sem = ctx.enter_context(nc.semaphore("pe"))
        dve_sem = ctx.enter_context(nc.semaphore("dve"))
        store_sem = ctx.enter_context(nc.semaphore("store"))
        all_sems = csem + [bias_sem, pe_sem, dve_sem, store_sem]

        # ---- loads: c0 first (input layer), HWDGE on sync for c0-c3,
        # SWDGE on gpsimd for the late chunks c4, c5 (parallel issue) ----
        def load(eng, i):
            c0, c1 = chunks[i], chunks[i + 1]
            eng.dma_start(out=W[:, c0:c1], in_=packs[i][:]).then_inc(csem[i], 16)

        load(nc.sync, 0)
        nc.sync.dma_start(out=Bt[:], in_=biasd[:]).then_inc(bias_sem, 16)
        for i in (1, 2, 3):
            load(nc.sync, i)
        for i in (4, 5):
            load(nc.gpsimd, i)

        # ---- ACT: preload the sigmoid table set early (off critical path) --
        nc.scalar.activation(
            scratch[:], scratch[:], mybir.ActivationFunctionType.Sigmoid
        )

        # ---- PE program -------------------------------------------------
        pe = nc.tensor
        peng = nc.engines[mybir.EngineType.PE]

        def mm(out, lhsT, rhs, start, stop, inc=False):
            m = pe.matmul(out, lhsT, rhs, start=start, stop=stop)
            if inc:
                m.then_inc(pe_sem, 1)

        peng.wait_ge(csem[0], 16)
        for m in range(2):
            r = slice(64 * m, 64 * (m + 1))
            mm(A[m][:], W[r, OFF_WIN : OFF_WIN + 128], W[r, 0:BPC],
               True, True, inc=True)
        hh = h0
        for l in range(3):
            out = (g1, g2, g3)[l]
            c0 = OFF_GAT + (2 * l) * 256
            c1 = OFF_GAT + (2 * l + 1) * 256
            bank = [A[(2 + 2 * l) % 4], A[(3 + 2 * l) % 4]]
            peng.wait_ge(csem[1 + l], 16)
            peng.wait_ge(dve_sem, 2 * l + 1)          # hh[0] ready
            for m in range(2):
                mm(bank[m][:], W[:, c0 + m * 128 : c0 + m * 128 + 128],
                   hh[0][:], True, False)
            peng.wait_ge(dve_sem, 2 * l + 2)          # hh[1] ready
            for m in range(2):
                mm(bank[m][:], W[:, c1 + m * 128 : c1 + m * 128 + 128],
                   hh[1][:], False, True, inc=True)
            hh = out
        peng.wait_ge(csem[4], 16)
        peng.wait_ge(dve_sem, 7)
        for m in range(2):
            mm(A[m][:], W[:, OFF_FUSE + m * 128 : OFF_FUSE + m * 128 + 128],
               hh[0][:], True, False)
        peng.wait_ge(dve_sem, 8)
        for m in range(2):
            mm(A[m][:], W[:, OFF_FUSE + 256 + m * 128 : OFF_FUSE + 256 + m * 128 + 128],
               hh[1][:], False, False)
        for m in range(2):
            r = slice(64 * m, 64 * (m + 1))
            mm(A[m][:], W[r, OFF_FUSE2 : OFF_FUSE2 + 128], W[r, BPC : 2 * BPC],
               False, True, inc=True)
        peng.wait_ge(csem[5], 16)
        peng.wait_ge(dve_sem, 9)                      # fu[0]
        mm(A[2][:], W[:, OFF_P1 : OFF_P1 + 128], fu[0][:], True, False)
        mm(A[3][:], W[:, OFF_D1 : OFF_D1 + 128], fu[0][:], True, False)
        peng.wait_ge(dve_sem, 10)                     # fu[1]
        mm(A[2][:], W[:, OFF_P1 + 128 : OFF_P1 + 256], fu[1][:], False, True, inc=True)
        mm(A[3][:], W[:, OFF_D1 + 128 : OFF_D1 + 256], fu[1][:], False, True, inc=True)
        peng.wait_ge(dve_sem, 11)
        mm(Pp2[:], W[:, OFF_P2 : OFF_P2 + 64], a1p[:], True, True, inc=True)
        peng.wait_ge(dve_sem, 12)
        mm(Pd2[:], W[:, OFF_D2 : OFF_D2 + 64], a1d[:], True, True, inc=True)
        peng.wait_ge(dve_sem, 13)
        mm(Pp3[:], W[0:64, OFF_P3 : OFF_P3 + 1], a2p[:], True, True, inc=True)
        peng.wait_ge(dve_sem, 14)
        mm(Pd3[:], W[0:64, OFF_D3 : OFF_D3 + 1], a2d[:], True, True, inc=True)

        # ---- DVE program ------------------------------------------------
        veng = nc.engines[mybir.EngineType.DVE]

        def rb(out, psum, j, pe_need, r1=128):
            veng.wait_ge(pe_sem, pe_need)
            nc.vector.tensor_scalar(
                out, psum, Bt[0:r1, j : j + 1], 0.0, ADD, MAX
            ).then_inc(dve_sem, 1)

        veng.wait_ge(bias_sem, 16)
        rb(h0[0][:], A[0][:], 0, 1)
        rb(h0[1][:], A[1][:], 1, 2)
        for l, out in enumerate((g1, g2, g3)):
            for m in range(2):
                rb(out[m][:], A[(2 + 2 * l + m) % 4][:], 2 + 2 * l + m,
                   3 + 2 * l + m)
        rb(fu[0][:], A[0][:], 8, 9)
        rb(fu[1][:], A[1][:], 9, 10)
        rb(a1p[:], A[2][:], 10, 11)
        rb(a1d[:], A[3][:], 11, 12)
        rb(a2p[:], Pp2[:], 12, 13, r1=64)
        rb(a2d[:], Pd2[:], 13, 14, r1=64)
        veng.wait_ge(pe_sem, 15)
        nc.vector.tensor_scalar(
            price[:], Pp3[:], Bt[0:1, 14:15], None, ADD
        ).then_inc(dve_sem, 1)

        # ---- ACT: real sigmoid + dir store (scalar is HWDGE-capable) ----
        aeng = nc.engines[mybir.EngineType.Activation]
        aeng.wait_ge(pe_sem, 16)
        nc.scalar.activation(
            dirn[:], Pd3[:], mybir.ActivationFunctionType.Sigmoid,
            bias=Bt[0:1, 15:16],
        )
        nc.scalar.dma_start(out=out_d[1:2, :], in_=dirn[:]).then_inc(store_sem, 16)

        # ---- sync: price store ------------------------------------------
        seng = nc.engines[mybir.EngineType.SP]
        seng.wait_ge(dve_sem, 15)
        nc.sync.dma_start(out=out_d[0:1, :], in_=price[:]).then_inc(store_sem, 16)

        # ---- gpsimd: sole final waiter; clear sems for re-execution -----
        geng = nc.engines[mybir.EngineType.Pool]
        geng.wait_ge(store_sem, 32)
        nums = sorted(s.num for s in all_sems)
        from concourse._compat import not_none
        lo = nums[0]
        hi = nums[-1]
        assert nums == list(range(lo, hi + 1)), nums
        nc.gpsimd.dma_reset(range(lo, hi + 1))
        nc.gpsimd.sem_clear(range(lo, hi + 1))

    return nc


def _build_nc_tile():
    import concourse.bass as bass
    import concourse.mybir as mybir
    import concourse.tile as tile
    from concourse import bacc

    f32 = mybir.dt.float32
    f16 = mybir.dt.float16 if USE_F16 else mybir.dt.float32
    ADD = mybir.AluOpType.add
    MAX = mybir.AluOpType.max

    # Bacc (not raw Bass): its compile() splits multi-sem waits into event
    # semaphores, which TRN2's one-wait-per-instruction codegen requires.
    nc = bacc.Bacc("TRN2", target_bir_lowering=False, debug=False,
                   num_devices=N_CORES)
    pack = nc.declare_dram_parameter("pack", [128, COLS], f16, isOutput=False)
    biasd = nc.declare_dram_parameter("biasp", [128, 16], f32, isOutput=False)
    out_d = nc.declare_dram_parameter("out", [2, BPC], f32, isOutput=True)

    with tile.TileContext(nc) as tc:
        with (
            tc.tile_pool(name="w", bufs=1) as wp,
            tc.tile_pool(name="a", bufs=1) as ab,
            tc.tile_pool(name="ps", bufs=2, space=bass.MemorySpace.PSUM) as pp,
        ):
            W = wp.tile([128, COLS], f16, tag="W", name="W")
            Bt = wp.tile([128, 16], f32, tag="Bt", name="Bt")
            # chunked loads, ordered by consumption, so compute overlaps DMA
            chunks = [0, OFF_GAT, OFF_GAT + 512, OFF_GAT + 1024, OFF_FUSE,
                      OFF_P1, COLS]
            nc.sync.dma_start(out=Bt[:], in_=biasd[:])
            for c0, c1 in zip(chunks[:-1], chunks[1:]):
                nc.sync.dma_start(out=W[:, c0:c1], in_=pack[:, c0:c1])

            G = [W[:, OFF_GAT + i * 256 : OFF_GAT + (i + 1) * 256] for i in range(6)]
            F0 = W[:, OFF_FUSE : OFF_FUSE + 256]
            F1 = W[:, OFF_FUSE + 256 : OFF_FUSE + 512]
            F2 = W[:, OFF_FUSE2 : OFF_FUSE2 + 128]
            P1 = W[:, OFF_P1 : OFF_P1 + 256]
            D1 = W[:, OFF_D1 : OFF_D1 + 256]
            TAIL = W[:, OFF_P2:COLS]  # p2|d2|p3|d3

            def bias(j, r0=0, r1=128):
                return Bt[r0:r1, j : j + 1]

            def relu_bias(ps, j, shape, tag, r1=128):
                t = ab.tile(shape, f16, tag=tag, name=tag)
                nc.vector.tensor_scalar(t[:], ps[:], bias(j, 0, r1), 0.0, ADD, MAX)
                return t

            # input layer: h[m] = relu(W_in[m-tile] @ xT + b_in)
            h = []
            for m in range(2):
                r = slice(64 * m, 64 * (m + 1))
                ps = pp.tile([128, BPC], f32, tag="ps", name="ps")
                nc.tensor.matmul(ps[:], W[r, OFF_WIN : OFF_WIN + 128],
                                 W[r, 0:BPC], start=True, stop=True)
                h.append(relu_bias(ps, m, [128, BPC], f"h0_{m}"))

            # 3 collapsed GAT layers
            for l in range(3):
                nh = []
                for m in range(2):
                    ms = slice(m * 128, (m + 1) * 128)
                    ps = pp.tile([128, BPC], f32, tag="ps", name="ps")
                    nc.tensor.matmul(ps[:], G[2 * l][:, ms], h[0][:], start=True, stop=False)
                    nc.tensor.matmul(ps[:], G[2 * l + 1][:, ms], h[1][:], start=False, stop=True)
                    nh.append(relu_bias(ps, 2 + 2 * l + m, [128, BPC], f"h{l + 1}_{m}"))
                h = nh

            # fuse layer: concat([h, embT]) @ W_fuse.T
            f = []
            for m in range(2):
                ms = slice(m * 128, (m + 1) * 128)
                r = slice(64 * m, 64 * (m + 1))
                ps = pp.tile([128, BPC], f32, tag="ps", name="ps")
                nc.tensor.matmul(ps[:], F0[:, ms], h[0][:], start=True, stop=False)
                nc.tensor.matmul(ps[:], F1[:, ms], h[1][:], start=False, stop=False)
                nc.tensor.matmul(ps[:], F2[r, :], W[r, BPC : 2 * BPC], start=False, stop=True)
                f.append(relu_bias(ps, 8 + m, [128, BPC], f"f_{m}"))

            # heads: 256 -> 128 -> 64 -> 1
            def head(W1, c2, c3, j1, j2, j3, sigmoid):
                ps = pp.tile([128, BPC], f32, tag="ps", name="ps")
                nc.tensor.matmul(ps[:], W1[:, 0:128], f[0][:], start=True, stop=False)
                nc.tensor.matmul(ps[:], W1[:, 128:256], f[1][:], start=False, stop=True)
                a1 = relu_bias(ps, j1, [128, BPC], f"a1_{j1}")
                ps2 = pp.tile([64, BPC], f32, tag="ps2", name="ps2")
                nc.tensor.matmul(ps2[:], TAIL[:, c2 : c2 + 64], a1[:], start=True, stop=True)
                a2 = relu_bias(ps2, j2, [64, BPC], f"a2_{j2}", r1=64)
                ps3 = pp.tile([1, BPC], f32, tag="ps3", name="ps3")
                nc.tensor.matmul(ps3[:], TAIL[0:64, c3 : c3 + 1], a2[:], start=True, stop=True)
                resn = f"res_{j3}"
                res = ab.tile([1, BPC], f32, tag=resn, name=resn)
                if sigmoid:
                    nc.scalar.activation(
                        res[:], ps3[:],
                        mybir.ActivationFunctionType.Sigmoid, bias=bias(j3, 0, 1),
                    )
                else:
                    nc.vector.tensor_scalar(res[:], ps3[:], bias(j3, 0, 1), None, ADD)
                return res

            price = head(P1, 0, 128, 10, 12, 14, sigmoid=False)
            dirn = head(D1, 64, 129, 11, 13, 15, sigmoid=True)
            # two independent stores on different DMA engines so they overlap
            nc.sync.dma_start(out=out_d[0:1, :], in_=price[:])
            nc.gpsimd.dma_start(out=out_d[1:2, :], in_=dirn[:])

    nc.compile()
    return nc


def _pack_host(inputs):
    f32 = lambda k: np.ascontiguousarray(np.asarray(inputs[k], dtype=np.float32))
    W_in, b_in = f32("W_in"), f32("b_in")
    gat_W, gat_b = f32("gat_W"), f32("gat_b")
    W_fuse, b_fuse = f32("W_fuse"), f32("b_fuse")
    W_p1, b_p1 = f32("W_p1"), f32("b_p1")
    W_p2, b_p2 = f32("W_p2"), f32("b_p2")
    W_p3, b_p3 = f32("W_p3"), f32("b_p3")
    W_d1, b_d1 = f32("W_d1"), f32("b_d1")
    W_d2, b_d2 = f32("W_d2"), f32("b_d2")
    W_d3, b_d3 = f32("W_d3"), f32("b_d3")

    bias = np.zeros((128, 16), np.float32)
    bias[:, 0], bias[:, 1] = b_in[:128], b_in[128:]
    for l in range(3):
        for m in range(2):
            bias[:, 2 + 2 * l + m] = gat_b[l, 128 * m : 128 * (m + 1)]
    bias[:, 8], bias[:, 9] = b_fuse[:128], b_fuse[128:]
    bias[:, 10], bias[:, 11] = b_p1, b_d1
    bias[:64, 12], bias[:64, 13] = b_p2, b_d2
    bias[0, 14], bias[0, 15] = b_p3[0], b_d3[0]

    np16 = np.float16 if USE_F16 else np.float32
    pk = np.zeros((128, COLS), np16)
    WinT = W_in.T.astype(np16)  # [64, 256]
    pk[0:64, OFF_WIN : OFF_WIN + 128] = WinT[:, 0:128]
    pk[64:128, OFF_WIN : OFF_WIN + 128] = WinT[:, 128:256]
    for l in range(3):
        GT = gat_W[l].T.astype(np16)  # [256, 256]
        for k in range(2):
            c = OFF_GAT + (2 * l + k) * 256
            pk[:, c : c + 256] = GT[128 * k : 128 * (k + 1), :]
    FT = W_fuse.T.astype(np16)  # [320, 256]
    pk[:, OFF_FUSE : OFF_FUSE + 256] = FT[0:128]
    pk[:, OFF_FUSE + 256 : OFF_FUSE + 512] = FT[128:256]
    pk[0:64, OFF_FUSE2 : OFF_FUSE2 + 128] = FT[256:320, 0:128]
    pk[64:128, OFF_FUSE2 : OFF_FUSE2 + 128] = FT[256:320, 128:256]
    for W1, off in ((W_p1, OFF_P1), (W_d1, OFF_D1)):
        T = W1.T.astype(np16)  # [256, 128]
        pk[:, off : off + 128] = T[0:128]
        pk[:, off + 128 : off + 256] = T[128:256]
    pk[:, OFF_P2 : OFF_P2 + 64] = W_p2.T.astype(np16)
    pk[:, OFF_D2 : OFF_D2 + 64] = W_d2.T.astype(np16)
    pk[0:64, OFF_P3] = W_p3[0].astype(np16)
    pk[0:64, OFF_D3] = W_d3[0].astype(np16)
    return pk, bias


def _build_in_maps(inputs):
    x = np.asarray(inputs["x"], dtype=np.float32)
    ci = np.asarray(inputs["company_indices"]).astype(np.int64)
    emb = np.asarray(inputs["emb"], dtype=np.float32)
    comp_emb = emb[ci]  # [B, 64]

    base, bias = _pack_host(inputs)
    chunks = [0, OFF_GAT, OFF_GAT + 512, OFF_GAT + 1024, OFF_FUSE, OFF_P1, COLS]
    in_maps = []
    for c in range(N_CORES):
        pk = base.copy()
        rows = slice(c * BPC, (c + 1) * BPC)
        xT = x[rows].T.astype(base.dtype)  # [64, BPC]
        eT = comp_emb[rows].T.astype(base.dtype)
        pk[0:64, 0:BPC] = xT
        pk[64:128, 0:BPC] = xT
        pk[0:64, BPC : 2 * BPC] = eT
        pk[64:128, BPC : 2 * BPC] = eT
        m = {
            f"pack{i}": np.ascontiguousarray(pk[:, c0:c1])
            for i, (c0, c1) in enumerate(zip(chunks[:-1], chunks[1:]))
        }
        m["biasp"] = bias
        in_maps.append(m)
    return in_maps


def kernel(**inputs):
    if "nc" not in _CACHE:
        _CACHE["nc"] = _build_nc()
    nc = _CACHE["nc"]
    from concourse.bass_utils import run_bass_kernel_spmd

    in_maps = _build_in_maps(inputs)
    res = run_bass_kernel_spmd(nc, in_maps, list(range(N_CORES)))
    outs = res.results
    price = np.concatenate([outs[c]["out"][0] for c in range(N_CORES)]).astype(np.float32)
    direction = np.concatenate([outs[c]["out"][1] for c in range(N_CORES)]).astype(np.float32)
    return price, direction


# revision 16
# speedup vs baseline: 1.1860x; 1.1083x over previous
"""Trainium2 Bass kernel for nn_EnhancedFinancialGAT.

Mathematical collapse: the reference broadcasts each batch item's feature
vector to all N=2000 graph nodes, so every node starts identical. A GAT
layer on identical node features returns, for every node, the attention-
weighted average of identical projected vectors -- and per-dst softmax
weights sum to exactly 1 in f32 (denom + 1e-16 == denom), so each layer
reduces to relu(h @ W.T + b). Every node stays identical through all 3
layers, and the company-node gather picks that shared vector. The whole
model is therefore an MLP:

  h = relu(x @ W_in.T + b_in)
  h = relu(h @ gat_W[l].T + gat_b[l])   for l in 0..2
  fused = relu(concat([h, emb[company_indices]]) @ W_fuse.T + b_fuse)
  price = W_p3 @ relu(W_p2 @ relu(W_p1 @ fused + b_p1) + b_p2) + b_p3
  direction = sigmoid(same with d-weights)

Verified numerically: collapsed-vs-full relative error ~2e-7 (pure f32
rounding noise of the softmax-weighted sums). The whole kernel is exact
f32: end-to-end relative error vs the reference is ~6e-7.

Sharding: data-parallel over batch (64 rows -> 8 rows/core). Weights are
replicated, pre-transposed on host into per-chunk contiguous DRAM
tensors (activations-transposed layout [feature, batch], so no on-device
transposes are needed anywhere).

Raw Bass program (no TileContext, no Block): everything lives in the
single entry basic block -- no branches (so no IRAM I$-miss stalls) and
no all-engine barrier waves beyond the framework's fixed preamble.
Loads are spread over the three DMA-capable engine queues (sync and
scalar are HWDGE, gpsimd is SWDGE) ordered by when their consumer needs
them, so the fp32 matmul chain starts as soon as chunk0 lands and rarely
waits on DMA again. The sigmoid ACT table set is preloaded via a dummy
activation while weights stream in. PSUM: 4 rotating [128,8] banks for
the wide groups (WAR is covered by the RAW waits since DVE is in-order)
+ 4 dedicated banks for the head tails = exactly 8 banks.
"""

import numpy as np

USE_F16 = False  # fp32 is exact (6e-7); f16 lands ~1e-3 (PE computes 16-bit at bf16 precision)

B = 64
N_CORES = 8
BPC = B // N_CORES  # batch rows per core

# -------- packed [128, COLS] layout (column offsets) --------
OFF_ACTS = 0                       # 16: xT 0:8, embT 8:16 (rows 0:64, dup to 64:128)
OFF_BIAS = 16                      # 16: see bias map below
OFF_WIN = 32                       # 128: rows 0:64 -> M[0:128], rows 64:128 -> M[128:256]
OFF_GAT = 176                      # 6 blocks of 256: block (l,k) at (2l+k)*256
OFF_FUSE = OFF_GAT + 6 * 256       # k0 [128,256], k1 [128,256]
OFF_FUSE2 = OFF_FUSE + 512         # k2 row-split, 128 cols
OFF_P1 = OFF_FUSE2 + 128           # k0 cols 0:128, k1 cols 128:256
OFF_D1 = OFF_P1 + 256
OFF_P2 = OFF_D1 + 256              # [128, 64]
OFF_D2 = OFF_P2 + 64
OFF_P3 = OFF_D2 + 64               # rows 0:64, 1 col
OFF_D3 = OFF_P3 + 1
COLS = OFF_D3 + 1                  # 2994

# bias columns (OFF_BIAS + j):
#  0,1: b_in | 2..7: gat_b (l,m) | 8,9: b_fuse | 10: b_p1 | 11: b_d1
#  12: b_p2 (rows 0:64) | 13: b_d2 | 14 row0: b_p3 | 15 row0: b_d3

# chunk boundaries: c0 = acts+bias+w_in, c1/c2/c3 = gat layers,
# c4 = fuse, c5 = heads
CHUNKS = [0, OFF_GAT, OFF_GAT + 512, OFF_GAT + 1024, OFF_FUSE, OFF_P1, COLS]

_CACHE = {}


def _build_nc():
    from contextlib import ExitStack

    import concourse.bass as bass
    import concourse.mybir as mybir

    f32 = mybir.dt.float32
    f16 = mybir.dt.float16 if USE_F16 else mybir.dt.float32
    ADD = mybir.AluOpType.add
    MAX = mybir.AluOpType.max

    nc = bass.Bass("TRN2", debug=False, num_devices=N_CORES)
    # one contiguous DRAM tensor per chunk: strided slices of a single
    # wide tensor DMA slowly; contiguous blocks run at queue line rate
    packs = [
        nc.declare_dram_parameter(f"pack{i}", [128, c1 - c0], f16, isOutput=False)
        for i, (c0, c1) in enumerate(zip(CHUNKS[:-1], CHUNKS[1:]))
    ]
    out_d = nc.declare_dram_parameter("out", [2, BPC], f32, isOutput=True)

    ctx = ExitStack()
    with ctx:
        sb = lambda nm, shape, dt: ctx.enter_context(nc.sbuf_tensor(nm, shape, dt))
        psb = lambda nm, shape: ctx.enter_context(nc.psum_tensor(nm, shape, f32))
        W = sb("W", [128, COLS], f16)
        h0 = [sb(f"h0_{i}", [128, BPC], f16) for i in range(2)]
        g1 = [sb(f"g1_{i}", [128, BPC], f16) for i in range(2)]
        g2 = [sb(f"g2_{i}", [128, BPC], f16) for i in range(2)]
        g3 = [sb(f"g3_{i}", [128, BPC], f16) for i in range(2)]
        fu = [sb(f"fu_{i}", [128, BPC], f16) for i in range(2)]
        a1p = sb("a1p", [128, BPC], f16)
        a1d = sb("a1d", [128, BPC], f16)
        a2p = sb("a2p", [64, BPC], f16)
        a2d = sb("a2d", [64, BPC], f16)
        price = sb("price", [1, BPC], f32)
        dirn = sb("dirn", [1, BPC], f32)
        scratch = sb("scratch", [1, BPC], f32)
        A = [psb(f"A{i}", [128, BPC]) for i in range(4)]
        Pp2 = psb("Pp2", [64, BPC])
        Pd2 = psb("Pd2", [64, BPC])
        Pp3 = psb("Pp3", [1, BPC])
        Pd3 = psb("Pd3", [1, BPC])

        csem = [ctx.enter_context(nc.semaphore(f"c{i}")) for i in range(6)]
        pe_sem = ctx.enter_context(nc.semaphore("pe"))
        dve_sem = ctx.enter_context(nc.semaphore("dve"))
        store_sem = ctx.enter_context(nc.semaphore("store"))
        all_sems = csem + [pe_sem, dve_sem, store_sem]

        # ---- loads: three parallel engine queues, ordered by need time.
        # Per-queue transfers serialize, so each stream is ordered by its
        # consumers: sync c0 (input layer) then gat1; scalar gat0 (+ ACT
        # table preload after) then fuse; gpsimd gat2 then heads.
        def load(eng, i):
            c0, c1 = CHUNKS[i], CHUNKS[i + 1]
            eng.dma_start(out=W[:, c0:c1], in_=packs[i][:]).then_inc(csem[i], 16)

        load(nc.sync, 0)
        load(nc.scalar, 1)
        load(nc.sync, 2)
        load(nc.gpsimd, 3)
        # preload the sigmoid table set early (off the critical path)
        nc.scalar.activation(
            scratch[:], scratch[:], mybir.ActivationFunctionType.Sigmoid
        )
        load(nc.scalar, 4)
        load(nc.gpsimd, 5)

        def bias(j, r1=128, r0=0):
            return W[r0:r1, OFF_BIAS + j : OFF_BIAS + j + 1]

        # ---- PE program -------------------------------------------------
        pe = nc.tensor
        peng = nc.engines[mybir.EngineType.PE]

        def mm(out, lhsT, rhs, start, stop, inc=False):
            m = pe.matmul(out, lhsT, rhs, start=start, stop=stop)
            if inc:
                m.then_inc(pe_sem, 1)

        peng.wait_ge(csem[0], 16)
        for m in range(2):
            r = slice(64 * m, 64 * (m + 1))
            mm(A[m][:], W[r, OFF_WIN : OFF_WIN + 128], W[r, 0:BPC],
               True, True, inc=True)
        hh = h0
        for l in range(3):
            out = (g1, g2, g3)[l]
            c0 = OFF_GAT + (2 * l) * 256
            c1 = OFF_GAT + (2 * l + 1) * 256
            bank = [A[(2 + 2 * l) % 4], A[(3 + 2 * l) % 4]]
            peng.wait_ge(csem[1 + l], 16)
            peng.wait_ge(dve_sem, 2 * l + 1)          # hh[0] ready
            for m in range(2):
                mm(bank[m][:], W[:, c0 + m * 128 : c0 + m * 128 + 128],
                   hh[0][:], True, False)
            peng.wait_ge(dve_sem, 2 * l + 2)          # hh[1] ready
            for m in range(2):
                mm(bank[m][:], W[:, c1 + m * 128 : c1 + m * 128 + 128],
                   hh[1][:], False, True, inc=True)
            hh = out
        peng.wait_ge(csem[4], 16)
        peng.wait_ge(dve_sem, 7)
        for m in range(2):
            mm(A[m][:], W[:, OFF_FUSE + m * 128 : OFF_FUSE + m * 128 + 128],
               hh[0][:], True, False)
        peng.wait_ge(dve_sem, 8)
        for m in range(2):
            mm(A[m][:], W[:, OFF_FUSE + 256 + m * 128 : OFF_FUSE + 256 + m * 128 + 128],
               hh[1][:], False, False)
        for m in range(2):
            r = slice(64 * m, 64 * (m + 1))
            mm(A[m][:], W[r, OFF_FUSE2 : OFF_FUSE2 + 128], W[r, BPC : 2 * BPC],
               False, True, inc=True)
        peng.wait_ge(csem[5], 16)
        peng.wait_ge(dve_sem, 9)                      # fu[0]
        mm(A[2][:], W[:, OFF_P1 : OFF_P1 + 128], fu[0][:], True, False)
        mm(A[3][:], W[:, OFF_D1 : OFF_D1 + 128], fu[0][:], True, False)
        peng.wait_ge(dve_sem, 10)                     # fu[1]
        mm(A[2][:], W[:, OFF_P1 + 128 : OFF_P1 + 256], fu[1][:], False, True, inc=True)
        mm(A[3][:], W[:, OFF_D1 + 128 : OFF_D1 + 256], fu[1][:], False, True, inc=True)
        peng.wait_ge(dve_sem, 11)
        mm(Pp2[:], W[:, OFF_P2 : OFF_P2 + 64], a1p[:], True, True, inc=True)
        peng.wait_ge(dve_sem, 12)
        mm(Pd2[:], W[:, OFF_D2 : OFF_D2 + 64], a1d[:], True, True, inc=True)
        peng.wait_ge(dve_sem, 13)
        mm(Pp3[:], W[0:64, OFF_P3 : OFF_P3 + 1], a2p[:], True, True, inc=True)
        peng.wait_ge(dve_sem, 14)
        mm(Pd3[:], W[0:64, OFF_D3 : OFF_D3 + 1], a2d[:], True, True, inc=True)

        # ---- DVE program ------------------------------------------------
        veng = nc.engines[mybir.EngineType.DVE]

        def rb(out, psum, j, pe_need, r1=128):
            veng.wait_ge(pe_sem, pe_need)
            nc.vector.tensor_scalar(
                out, psum, bias(j, r1), 0.0, ADD, MAX
            ).then_inc(dve_sem, 1)

        rb(h0[0][:], A[0][:], 0, 1)
        rb(h0[1][:], A[1][:], 1, 2)
        for l, out in enumerate((g1, g2, g3)):
            for m in range(2):
                rb(out[m][:], A[(2 + 2 * l + m) % 4][:], 2 + 2 * l + m,
                   3 + 2 * l + m)
        rb(fu[0][:], A[0][:], 8, 9)
        rb(fu[1][:], A[1][:], 9, 10)
        rb(a1p[:], A[2][:], 10, 11)
        rb(a1d[:], A[3][:], 11, 12)
        rb(a2p[:], Pp2[:], 12, 13, r1=64)
        rb(a2d[:], Pd2[:], 13, 14, r1=64)
        veng.wait_ge(pe_sem, 15)
        nc.vector.tensor_scalar(
            price[:], Pp3[:], bias(14, 1), None, ADD
        ).then_inc(dve_sem, 1)

        # ---- ACT: real sigmoid + dir store (scalar is HWDGE-capable) ----
        aeng = nc.engines[mybir.EngineType.Activation]
        aeng.wait_ge(pe_sem, 16)
        nc.scalar.activation(
            dirn[:], Pd3[:], mybir.ActivationFunctionType.Sigmoid,
            bias=bias(15, 1),
        )
        nc.scalar.dma_start(out=out_d[1:2, :], in_=dirn[:]).then_inc(store_sem, 16)

        # ---- sync: price store ------------------------------------------
        seng = nc.engines[mybir.EngineType.SP]
        seng.wait_ge(dve_sem, 15)
        nc.sync.dma_start(out=out_d[0:1, :], in_=price[:]).then_inc(store_sem, 16)

        # ---- gpsimd: sole final waiter; clear sems for re-execution -----
        geng = nc.engines[mybir.EngineType.Pool]
        geng.wait_ge(store_sem, 32)
        nums = sorted(s.num for s in all_sems)
        lo, hi = nums[0], nums[-1]
        assert nums == list(range(lo, hi + 1)), nums
        nc.gpsimd.dma_reset(range(lo, hi + 1))
        nc.gpsimd.sem_clear(range(lo, hi + 1))

    return nc


def _pack_host(inputs):
    f32 = lambda k: np.ascontiguousarray(np.asarray(inputs[k], dtype=np.float32))
    W_in, b_in = f32("W_in"), f32("b_in")
    gat_W, gat_b = f32("gat_W"), f32("gat_b")
    W_fuse, b_fuse = f32("W_fuse"), f32("b_fuse")
    W_p1, b_p1 = f32("W_p1"), f32("b_p1")
    W_p2, b_p2 = f32("W_p2"), f32("b_p2")
    W_p3, b_p3 = f32("W_p3"), f32("b_p3")
    W_d1, b_d1 = f32("W_d1"), f32("b_d1")
    W_d2, b_d2 = f32("W_d2"), f32("b_d2")
    W_d3, b_d3 = f32("W_d3"), f32("b_d3")

    np16 = np.float16 if USE_F16 else np.float32
    pk = np.zeros((128, COLS), np16)

    bias = pk[:, OFF_BIAS : OFF_BIAS + 16]
    bias[:, 0], bias[:, 1] = b_in[:128], b_in[128:]
    for l in range(3):
        for m in range(2):
            bias[:, 2 + 2 * l + m] = gat_b[l, 128 * m : 128 * (m + 1)]
    bias[:, 8], bias[:, 9] = b_fuse[:128], b_fuse[128:]
    bias[:, 10], bias[:, 11] = b_p1, b_d1
    bias[:64, 12], bias[:64, 13] = b_p2, b_d2
    bias[0, 14], bias[0, 15] = b_p3[0], b_d3[0]

    WinT = W_in.T.astype(np16)  # [64, 256]
    pk[0:64, OFF_WIN : OFF_WIN + 128] = WinT[:, 0:128]
    pk[64:128, OFF_WIN : OFF_WIN + 128] = WinT[:, 128:256]
    for l in range(3):
        GT = gat_W[l].T.astype(np16)  # [256, 256]
        for k in range(2):
            c = OFF_GAT + (2 * l + k) * 256
            pk[:, c : c + 256] = GT[128 * k : 128 * (k + 1), :]
    FT = W_fuse.T.astype(np16)  # [320, 256]
    pk[:, OFF_FUSE : OFF_FUSE + 256] = FT[0:128]
    pk[:, OFF_FUSE + 256 : OFF_FUSE + 512] = FT[128:256]
    pk[0:64, OFF_FUSE2 : OFF_FUSE2 + 128] = FT[256:320, 0:128]
    pk[64:128, OFF_FUSE2 : OFF_FUSE2 + 128] = FT[256:320, 128:256]
    for W1, off in ((W_p1, OFF_P1), (W_d1, OFF_D1)):
        T = W1.T.astype(np16)  # [256, 128]
        pk[:, off : off + 128] = T[0:128]
        pk[:, off + 128 : off + 256] = T[128:256]
    pk[:, OFF_P2 : OFF_P2 + 64] = W_p2.T.astype(np16)
    pk[:, OFF_D2 : OFF_D2 + 64] = W_d2.T.astype(np16)
    pk[0:64, OFF_P3] = W_p3[0].astype(np16)
    pk[0:64, OFF_D3] = W_d3[0].astype(np16)
    return pk


def _build_in_maps(inputs):
    x = np.asarray(inputs["x"], dtype=np.float32)
    ci = np.asarray(inputs["company_indices"]).astype(np.int64)
    emb = np.asarray(inputs["emb"], dtype=np.float32)
    comp_emb = emb[ci]  # [B, 64]

    base = _pack_host(inputs)
    in_maps = []
    for c in range(N_CORES):
        pk = base.copy()
        rows = slice(c * BPC, (c + 1) * BPC)
        xT = x[rows].T.astype(base.dtype)  # [64, BPC]
        eT = comp_emb[rows].T.astype(base.dtype)
        pk[0:64, 0:BPC] = xT
        pk[64:128, 0:BPC] = xT
        pk[0:64, BPC : 2 * BPC] = eT
        pk[64:128, BPC : 2 * BPC] = eT
        in_maps.append(
            {
                f"pack{i}": np.ascontiguousarray(pk[:, c0:c1])
                for i, (c0, c1) in enumerate(zip(CHUNKS[:-1], CHUNKS[1:]))
            }
        )
    return in_maps


def kernel(**inputs):
    if "nc" not in _CACHE:
        _CACHE["nc"] = _build_nc()
    nc = _CACHE["nc"]
    from concourse.bass_utils import run_bass_kernel_spmd

    in_maps = _build_in_maps(inputs)
    res = run_bass_kernel_spmd(nc, in_maps, list(range(N_CORES)))
    outs = res.results
    price = np.concatenate([outs[c]["out"][0] for c in range(N_CORES)]).astype(np.float32)
    direction = np.concatenate([outs[c]["out"][1] for c in range(N_CORES)]).astype(np.float32)
    return price, direction
